# revision 12
# baseline (speedup 1.0000x reference)
"""Trainium2 Bass kernel for nn_DenseBayesian (dense + hard LWTA grouped argmax mask).

Computes out = x @ W.T + b, then per group of U=4 output units keeps only the
argmax unit (others zeroed). Data-parallel over 8 NeuronCores along the row axis.

Numerics: the product runs entirely in fp16 (x and W rounded to fp16 on host;
fp16 x fp16 products accumulate exactly in fp32 PSUM). The grouped argmax is
a 2-round tournament in the fp16 domain, arranged so every DVE operand is
packed (stride-1 fp16): W's output columns are permuted to unit-major planes
(col' = u*128 + g), so the unit-j slices of the logit tile are contiguous.
Per macro the DVE then needs only TWO tensor_tensor ops over the same operand
pair: t = max(u02_planes, u13_planes) (runs in the DVE 2x fp16 mode) and
dpair = (u0-u2, u1-u3) stored as f8e5m2, whose sign bits are the pair-winner
flags (exact: fp16 subtract is sign-exact and f8 rounding preserves sign).
The host reconstructs winner value max(tAC,tBD) and index from the two signs;
+-0 gap bytes or tAC==tBD flag fp16-level ties (~4e-7 of groups), which the
host re-breaks with an exact fp32 recompute of those groups' logits. End-to-
end rel err ~1.09e-2 (winner flips only from the fp16 product rounding).

A windowed tensor_reduce (no DVE fast mode, 2x slower) and any broadcast-max
compare (stride-0 operand, also 1x) are avoided entirely; GpSimd is left idle
on purpose — Pool traffic stalls concurrent DVE 2x-mode ops (shared ports).

Output is compressed to pair-max fp16 [rows, 2, 128] + gap-sign f8
[rows, 2, 128] (24 MB/core-group vs 256 MB dense). Input is fp16 x (16 MB/
core). Per-core engine budget: PE ~150us (fp16-only matmul), Act ~126us
(PSUM->fp16 drain), DVE ~120us, DMA ~112us.

Self-contained: hardcodes the problem shapes; needs numpy + the concourse
runtime available on the host.
"""
import numpy as np

import concourse.bass as bass
import concourse.mybir as mybir
import concourse.tile as tile
from concourse import bacc
from concourse.bass_utils import run_bass_kernel_spmd

f32 = mybir.dt.float32
f16 = mybir.dt.float16
f8 = mybir.dt.float8e5
u8 = mybir.dt.uint8

N = 262144
DIN = 256
DOUT = 512
U = 4
G = DOUT // U               # groups (128)
NCORES = 8
ROWS = N // NCORES          # 32768 rows per core
MACRO = 512                 # rows per macro-tile (4 psum banks of 128 rows)
P = 128
KC = DIN // P               # k chunks (2)
NB = MACRO // P             # psum banks per macro (4)
XB = 2 * KC * MACRO         # input bytes/partition/macro (fp16 x: 2048)
TB = 2 * NB * 2 * G         # pair-max fp16 bytes/partition/macro (2048)
DB = 2 * NB * 2 * G         # gap fp16 bytes/partition/macro (2048)
OT = TB + DB                # 4096
ASPLIT = 1664               # logits drained by Act; DVE copies the rest

# output column permutation: col' = u*G + g holds unit u of group g
PERM = np.arange(DOUT).reshape(G, U).T.ravel()


def build_program(n_macros: int, with_bias: bool, mode: str = "stt"):
    assert mode == "stt"
    nc = bacc.Bacc("TRN2", target_bir_lowering=False)
    A = mybir.AluOpType
    AF = mybir.ActivationFunctionType

    xb_d = nc.dram_tensor("xb", [n_macros, P, XB], u8, kind="ExternalInput")
    wh_d = nc.dram_tensor("wh", [P, KC, DOUT], f16, kind="ExternalInput")
    if with_bias:
        bh_d = nc.dram_tensor("bh", [1, DOUT], f16, kind="ExternalInput")
        bl_d = nc.dram_tensor("bl", [1, DOUT], f16, kind="ExternalInput")
    out_d = nc.dram_tensor("out", [n_macros, P, OT], u8, kind="ExternalOutput")

    with tile.TileContext(nc) as tc:
        with tc.tile_pool(name="wpool", bufs=1) as wpool, \
             tc.tile_pool(name="xpool", bufs=6) as xpool, \
             tc.tile_pool(name="upool", bufs=4) as upool, \
             tc.tile_pool(name="opool", bufs=4) as opool, \
             tc.tile_pool(name="pspool", bufs=2, space="PSUM") as pspool:

            wh = wpool.tile([P, KC, DOUT], f16)
            nc.sync.dma_start(wh[:], wh_d[:])
            if with_bias:
                bh = wpool.tile([1, DOUT], f16)
                nc.sync.dma_start(bh[:], bh_d[:])
                bl = wpool.tile([1, DOUT], f16)
                nc.sync.dma_start(bl[:], bl_d[:])
                ones = wpool.tile([1, P], f16)
                nc.vector.memset(ones[:], 1.0)

            for mt in range(n_macros):
                xb = xpool.tile([P, XB], u8, tag="xb")
                nc.sync.dma_start(xb[:], xb_d[mt, :, :])
                xh_t = xb[:].bitcast(f16).rearrange("p (c m) -> p c m", c=KC)

                # two 2-bank PSUM tiles per macro: four independent slots
                # across the double-buffered pool, so the PE never stalls on
                # a full-macro drain and each Act copy starts after only two
                # banks stop
                ps_a = pspool.tile([P, 2 * DOUT], f32, tag="ps_a")
                ps_b = pspool.tile([P, 2 * DOUT], f32, tag="ps_b")
                for s in range(NB):
                    tgt = ps_a if s < 2 else ps_b
                    acc = tgt[:, (s % 2) * DOUT:(s % 2 + 1) * DOUT]
                    rs = slice(s * P, (s + 1) * P)
                    mms = []
                    if with_bias:
                        mms.append((ones[:, :], bh[:, :]))
                        mms.append((ones[:, :], bl[:, :]))
                    for c in range(KC):
                        mms.append((xh_t[:, c, rs], wh[:, c, :]))
                    last = len(mms) - 1
                    for i, (lhsT, rhs) in enumerate(mms):
                        nc.tensor.matmul(acc, lhsT, rhs,
                                         start=(i == 0), stop=(i == last))

                # fp16 copy of the logits (Act drains PSUM, one copy per tile)
                u16 = upool.tile([P, NB * DOUT], f16)
                nc.scalar.activation(u16[:, 0:1024], ps_a[:], AF.Copy)
                nc.scalar.activation(u16[:, 1024:], ps_b[:], AF.Copy)
                u16v = u16[:].rearrange("p (s u g) -> p s u g", u=U, g=G)
                in0 = u16v[:, :, 0:2, :]        # planes u0, u1 (packed)
                in1 = u16v[:, :, 2:4, :]        # planes u2, u3 (packed)

                # packed output tile: [pair-max fp16 2048B | gap fp16 2048B]
                # both TTs all-2-byte packed stride-1 => DVE 2x fp16 mode
                ot = opool.tile([P, OT], u8)
                tv = ot[:, 0:TB].bitcast(f16).rearrange(
                    "p (s c g) -> p s c g", c=2, g=G)
                dv = ot[:, TB:OT].bitcast(f16).rearrange(
                    "p (s c g) -> p s c g", c=2, g=G)
                nc.vector.tensor_tensor(tv, in0, in1, A.max)
                nc.vector.tensor_tensor(dv, in0, in1, A.subtract)

                nc.sync.dma_start(out_d[mt, :, :], ot[:])

    nc.compile()
    return nc


_programs: dict = {}


def _get_program(n_macros: int, with_bias: bool, mode: str = "stt"):
    key = (n_macros, with_bias, mode)
    if key not in _programs:
        _programs[key] = build_program(n_macros, with_bias, mode)
    return _programs[key]


def _tile_x(a: np.ndarray, n_macros: int):
    """[rows, DIN] -> [n_macros, P, KC, MACRO]: k = c*P + p, row = mt*MACRO + r."""
    at = np.ascontiguousarray(a.T)                      # [DIN, rows]
    at = at.reshape(KC, P, n_macros, MACRO)             # [c, p, mt, r]
    return np.ascontiguousarray(at.transpose(2, 1, 0, 3))


def _pack_x(xs: np.ndarray, n_macros: int):
    """[rows, DIN] fp32 -> packed u8 [n_macros, P, XB] (fp16)."""
    hi = xs.astype(np.float16)
    return _tile_x(hi, n_macros).view(np.uint8).reshape(n_macros, P, -1)


def _pack_w(W: np.ndarray):
    """[DOUT, DIN] fp32 -> fp16 W.T, columns permuted, tiled [P, KC, DOUT]."""
    wT = np.ascontiguousarray(W.astype(np.float32).T).astype(np.float16)
    wT = np.ascontiguousarray(wT[:, PERM])
    return np.ascontiguousarray(wT.reshape(KC, P, DOUT).transpose(1, 0, 2))


def _pack_b(b: np.ndarray):
    b32 = b.astype(np.float32).reshape(1, DOUT)[:, PERM]
    hi = b32.astype(np.float16)
    lo = (b32 - hi.astype(np.float32)).astype(np.float16)
    return hi, lo


def _in_maps(x, W, b, with_bias, n_macros):
    wh = _pack_w(W)
    maps = []
    for i in range(NCORES):
        xb = _pack_x(x[i * ROWS:(i + 1) * ROWS], n_macros)
        im = {"xb": xb, "wh": wh}
        if with_bias:
            bh, bl = _pack_b(b)
            im["bh"] = bh
            im["bl"] = bl
        maps.append(im)
    return maps


def _decode(outs: list[np.ndarray], x: np.ndarray, W: np.ndarray,
            b: np.ndarray, with_bias: bool) -> np.ndarray:
    """outs: per-core [n_macros, P, OT] u8 -> full [N, DOUT] f32."""
    o = np.stack(outs)                                   # [C, nm, P, OT]
    C, nm = o.shape[0], o.shape[1]
    # row = core*ROWS + mt*MACRO + s*P + p
    t = o[..., :TB].copy().view(np.float16).reshape(C, nm, P, NB, 2, G)
    t = t.transpose(0, 1, 3, 2, 4, 5).reshape(N, 2, G)
    dp = o[..., TB:].copy().view(np.float16).reshape(C, nm, P, NB, 2, G)
    dp = dp.transpose(0, 1, 3, 2, 4, 5).reshape(N, 2, G)

    tAC, tBD = t[:, 0], t[:, 1]                          # [N, G] fp16
    d02, d13 = dp[:, 0], dp[:, 1]                        # [N, G] fp16 gaps
    jAC = np.where(np.signbit(d02), 2, 0)                # sign(u0-u2)
    jBD = np.where(np.signbit(d13), 3, 1)                # sign(u1-u3)
    idx = np.where(tAC > tBD, jAC,
                   np.where(tBD > tAC, jBD, np.minimum(jAC, jBD)))
    vals = np.maximum(tAC, tBD)

    # fp16-level ties (gap == +-0 is exact in fp16): exact fp32 re-break
    flag = (tAC == tBD) | (d02 == 0) | (d13 == 0)
    tr, tg = np.nonzero(flag)
    if tr.size:
        xt = x[tr].astype(np.float16).astype(np.float32)
        Wg = W.astype(np.float16).astype(np.float32).reshape(G, U, DIN)[tg]
        lg = np.einsum("tk,tuk->tu", xt, Wg, optimize=True)
        if with_bias:
            lg = lg + b.reshape(G, U)[tg]
        idx[tr, tg] = lg.argmax(axis=1)

    out = np.zeros((N, G, U), np.float32)
    np.put_along_axis(out, idx[:, :, None],
                      vals[:, :, None].astype(np.float32), axis=2)
    return out.reshape(N, DOUT)


def kernel(x: np.ndarray, W: np.ndarray, b: np.ndarray) -> np.ndarray:
    x = np.asarray(x, dtype=np.float32)
    W = np.asarray(W, dtype=np.float32)
    b = np.asarray(b, dtype=np.float32)
    assert x.shape == (N, DIN) and W.shape == (DOUT, DIN) and b.shape == (DOUT,)

    with_bias = bool(np.any(b))
    n_macros = ROWS // MACRO
    nc = _get_program(n_macros, with_bias)
    maps = _in_maps(x, W, b, with_bias, n_macros)
    res = run_bass_kernel_spmd(nc, maps, list(range(NCORES)))
    return _decode([res.results[i]["out"] for i in range(NCORES)],
                   x, W, b, with_bias)
